# revision 1
# baseline (speedup 1.0000x reference)
"""Trainium2 Bass kernel for nn_Net_18906446037087 (snntorch Leaky SNN layer).

Reference semantics (per batch element, 255 steps, f32):
    cur = x @ W.T                         # [B, 1]
    m_0 = 0
    m_{t+1} = (0.95*m_t + cur) * (m_t <= 1)
    spk_{t+1} = (m_{t+1} > 1)
Outputs: (spk_rec, mem_rec), each [255, B, 1] f32.

Sharding: pure data parallel over batch across 8 cores (B=65536 -> 8192/core).

Numerics: the grading oracle runs jax on the axon/neuron backend. Its matmul
lowering is PE transpose + 7 K-chunk (6x128+16) fp32 matmuls (W stationary,
moving xT) accumulated in PSUM; its scan is plain f32 mul-then-add. Both are
reproduced bit-exactly here (verified empirically; x-stationary does NOT
bit-match because the PE fp32 two-pass split is weights-side). spk_rec is
derived on host as mem_rec > 1.0, which is exact.

Layout: per core, batch element e sits at membrane tile position [p, j]
with e = p*64 + j. Matmul group g handles columns j in [4g, 4g+4) via
row-strided x loads, so the scan over a column range can start as soon as
its groups finish: piece 0 (cols 0..PIECE1) scans on DVE while PE still
computes piece 1's matvec; the Tile scheduler interleaves piece 1's scan
ops into piece 0's dependent-issue stall slots on its own (manual
interleaving via CROSSOVER < 255 measured slightly worse).
Engine split: PE transposes+matmuls; PSUM->SBUF xT copies run on DVE for
piece 0's groups (DVE is idle before the scan starts and has faster PSUM
access than ACT) and on ACT for piece 1's groups (DVE is scanning by then);
DVE runs the scan; SP/sync all DMAs. cur is bounced to the partition-major
scan layout incrementally per group via a DRAM scratch.
"""
import sys
if "/opt/trn_rl_repo" not in sys.path:
    sys.path.insert(0, "/opt/trn_rl_repo")

import numpy as np
from contextlib import ExitStack

import concourse.bass as bass
import concourse.bacc as bacc
import concourse.mybir as mybir
import concourse.tile as tile
from concourse.bass_utils import run_bass_kernel_spmd

F32 = mybir.dt.float32
ALU = mybir.AluOpType

N_CORES = 8
B_FULL = 65536
B_CORE = B_FULL // N_CORES          # 8192
D = 784
NUM_STEPS = 255
BETA = 0.95
THRESHOLD = 1.0

GROUP = 512                          # batch rows per matmul group
NGROUP = B_CORE // GROUP             # 16
CHUNKS = [(0, 128), (128, 128), (256, 128), (384, 128), (512, 128), (640, 128), (768, 16)]

STAGE = 17                           # scan steps buffered per output DMA
NSTAGE = NUM_STEPS // STAGE          # 15
COLS = B_CORE // 128                 # 64 membrane-tile columns

# tunables
PIECE1 = 40                          # columns in piece 0 (rest in piece 1)
CROSSOVER = 255                      # piece-0 solo steps before interleaving
XG_BUFS = 2


def _build():
    nc = bacc.Bacc("TRN2", target_bir_lowering=False, debug=False,
                   num_devices=N_CORES)
    x_d = nc.dram_tensor("x", [B_CORE, D], F32, kind="ExternalInput")
    w_d = nc.dram_tensor("w", [128, 7], F32, kind="ExternalInput")
    id_d = nc.dram_tensor("ident", [128, 128], F32, kind="ExternalInput")
    mem_d = nc.dram_tensor("mem", [NUM_STEPS, B_CORE], F32, kind="ExternalOutput")
    curscratch_d = nc.dram_tensor("curscratch", [B_CORE], F32)

    pieces = [(0, PIECE1), (PIECE1, COLS - PIECE1)]

    # row view: x_rows[j][p] = x[p*64 + j]
    x_rows = x_d[:].rearrange("(p j) f -> j p f", j=COLS)

    with tile.TileContext(nc) as tc, ExitStack() as ctx:
        xpool = ctx.enter_context(tc.tile_pool(name="xpool", bufs=XG_BUFS))
        xtpool = ctx.enter_context(tc.tile_pool(name="xtpool", bufs=6))
        stpools = [
            ctx.enter_context(tc.tile_pool(name=f"stpool{i}", bufs=2))
            for i in range(len(pieces))
        ]
        const = ctx.enter_context(tc.tile_pool(name="const", bufs=1))
        psum = ctx.enter_context(tc.tile_pool(name="psum", bufs=4, space="PSUM"))
        psacc = ctx.enter_context(tc.tile_pool(name="psacc", bufs=2, space="PSUM"))

        w_t = const.tile([128, 7], F32)
        id_t = const.tile([128, 128], F32)
        nc.sync.dma_start(w_t[:], w_d[:])
        nc.sync.dma_start(id_t[:], id_d[:])

        cur_tiles = [
            const.tile([128, nc_], F32, name=f"cur{i}")
            for i, (_, nc_) in enumerate(pieces)
        ]
        cur_lines = [
            const.tile([1, nc_ * 128], F32, name=f"curline{i}")
            for i, (_, nc_) in enumerate(pieces)
        ]

        def matvec_group(g, pi, j0):
            """cur for batch columns [4g, 4g+4): strided x rows."""
            copy_eng = nc.vector.tensor_copy if pi == 0 else nc.scalar.copy
            xg = []
            for t in range(4):
                xt_ = xpool.tile([128, D], F32, tag=f"xg{t}")
                nc.sync.dma_start(xt_[:], x_rows[4 * g + t])
                xg.append(xt_)
            acc = psacc.tile([1, GROUP], F32, tag="acc")
            for ci, (c0, cl) in enumerate(CHUNKS):
                xt_ps = psum.tile([128, GROUP], F32, tag="xt")
                for t in range(4):
                    nc.tensor.transpose(
                        xt_ps[:cl, t * 128:(t + 1) * 128],
                        xg[t][:, c0:c0 + cl],
                        id_t[:],
                    )
                xt_sb = xtpool.tile([128, GROUP], F32, tag="xtsb")
                copy_eng(xt_sb[:cl, :], xt_ps[:cl, :])
                nc.tensor.matmul(
                    acc[:, :],
                    w_t[:cl, ci:ci + 1],
                    xt_sb[:cl, :],
                    start=(ci == 0),
                    stop=(ci == len(CHUNKS) - 1),
                )
            c = 4 * g - j0
            nc.scalar.copy(cur_lines[pi][:, c * 128:(c + 4) * 128], acc[:, :])
            sl = curscratch_d[(4 * g) * 128:(4 * g + 4) * 128]
            nc.sync.dma_start(sl, cur_lines[pi][:, c * 128:(c + 4) * 128])
            nc.sync.dma_start(
                cur_tiles[pi][:, c:c + 4],
                sl.rearrange("(c p) -> p c", p=128))

        class PieceScan:
            """Emits scan ops for one column piece, one step at a time."""

            def __init__(self, pi, j0, ncols):
                self.pi, self.j0, self.ncols = pi, j0, ncols
                self.t = 0
                self.mem_prev = None
                self.stage = None
                self.u = const.tile([128, ncols], F32, name=f"u{pi}")

            def step(self):
                pi, ncols = self.pi, self.ncols
                t = self.t
                assert t < NUM_STEPS
                s = t % STAGE
                if s == 0:
                    self.stage = stpools[pi].tile(
                        [128, STAGE * ncols], F32, tag=f"stage{pi}")
                sl = self.stage[:, s * ncols:(s + 1) * ncols]
                if t == 0:
                    nc.vector.tensor_copy(sl, cur_tiles[pi][:])
                else:
                    nc.vector.scalar_tensor_tensor(
                        self.u[:], self.mem_prev, BETA, cur_tiles[pi][:],
                        ALU.mult, ALU.add)
                    nc.vector.scalar_tensor_tensor(
                        sl, self.mem_prev, THRESHOLD, self.u[:],
                        ALU.is_le, ALU.mult)
                self.mem_prev = sl
                self.t = t + 1
                if s == STAGE - 1:
                    st = t // STAGE
                    j0 = self.j0
                    nc.sync.dma_start(
                        mem_d[st * STAGE:(st + 1) * STAGE, :]
                        .rearrange("s (p j) -> p s j", p=128)[:, :, j0:j0 + ncols],
                        self.stage[:].rearrange("p (s j) -> p s j", s=STAGE),
                    )

        scans = [PieceScan(pi, j0, nc_) for pi, (j0, nc_) in enumerate(pieces)]

        g = 0
        # piece 0 matvec
        for _ in range(pieces[0][1] // 4):
            matvec_group(g, 0, pieces[0][0])
            g += 1
        # piece 0 solo scan emission up to crossover; piece 1 matvec follows
        # in program order (PE/ACT run it concurrently with the DVE scan)
        for _ in range(min(CROSSOVER, NUM_STEPS)):
            scans[0].step()
        for _ in range(pieces[1][1] // 4):
            matvec_group(g, 1, pieces[1][0])
            g += 1
        # interleave remaining steps of both pieces
        while scans[0].t < NUM_STEPS or scans[1].t < NUM_STEPS:
            if scans[0].t < NUM_STEPS:
                scans[0].step()
            if scans[1].t < NUM_STEPS:
                scans[1].step()

    nc.compile()
    return nc


_NC_CACHE = None


def _get_nc():
    global _NC_CACHE
    if _NC_CACHE is None:
        _NC_CACHE = _build()
    return _NC_CACHE


def _prep_inputs(x, W):
    x = np.ascontiguousarray(np.asarray(x, dtype=np.float32))
    W = np.asarray(W, dtype=np.float32).reshape(-1)
    assert x.shape == (B_FULL, D) and W.shape == (D,)
    wpad = np.zeros(896, np.float32)
    wpad[:D] = W
    wcol = np.ascontiguousarray(wpad.reshape(7, 128).T)
    ident = np.eye(128, dtype=np.float32)
    in_maps = [
        {"x": x[d * B_CORE:(d + 1) * B_CORE], "w": wcol, "ident": ident}
        for d in range(N_CORES)
    ]
    return in_maps


def kernel(x, W, _trace=False, _trace_kwargs=None):
    nc = _get_nc()
    in_maps = _prep_inputs(x, W)
    res = run_bass_kernel_spmd(nc, in_maps, list(range(N_CORES)),
                               trace=_trace, **(_trace_kwargs or {}))
    mem = np.concatenate([res.results[d]["mem"] for d in range(N_CORES)], axis=1)
    mem_rec = mem.reshape(NUM_STEPS, B_FULL, 1)
    spk_rec = (mem_rec > np.float32(THRESHOLD)).astype(np.float32)
    if _trace:
        return (spk_rec, mem_rec), res
    return spk_rec, mem_rec



# revision 7
# speedup vs baseline: 1.7651x; 1.7651x over previous
"""Trainium2 Bass kernel for nn_Net_18906446037087 (snntorch Leaky SNN layer).

Reference semantics (per batch element, 255 steps, f32):
    cur = x @ W.T                         # [B, 1]
    m_0 = 0
    m_{t+1} = (0.95*m_t + cur) * (m_t <= 1)
    spk_{t+1} = (m_{t+1} > 1)
Outputs: (spk_rec, mem_rec), each [255, B, 1] f32.

Sharding: pure data parallel over batch across 8 cores (B=65536 -> 8192/core).

Closed form: the trajectory is periodic in t. With s[k] = (1-b^k)/(1-b),
an element first spikes at step K iff cur > 1/s[K]; then mem repeats the
pattern A_K[t] = s[((t-1) mod (K+1)) + 1] (0 at the reset slot); elements
with cur <= 1/s[255] follow the pure ramp R[t] = s[t]. So
    mem[t, b] = cur_b * (R[t] + sum_{k>=K(b)} (A_k - A_{k+1})[t])
which is one matmul  mem = G^T @ F  with
    G[0] = R, G[k] = A_k - A_{k+1} (A_256 := R)     (host-precomputed)
    F[k, b] = cur_b * [cur_b > theta_k], theta_0 = -inf, theta_k = 1/s[k].
F is built on-device from a PE broadcast of cur; the matmul runs in fp32r
(PE rounds operands to 11-bit mantissa; measured end-to-end rel err ~1e-3,
vs the 2e-2 gate). spk is derived on host as mem > 1.0.

Per-core pipeline (B_CORE=8192 = 16 groups of 512 = 64 subgroups of 128):
  per group: 4 contiguous x tiles [128,784] -> PE transposes (fp32, exact)
  -> PSUM->SBUF copies (DVE/ACT/Pool) -> 7 K-chunk matmuls with xT
  *stationary* and W moving (out [128,1] per subgroup, PSUM-accumulated;
  moving side is 1 wide so PE cost is nil) -> cur columns.
  per 4-group block: PE-transpose cur columns to rows, bounce 8KB through
  DRAM to get a [1,2048] row, PE ones-outer-product broadcasts each 512
  chunk to [128,512], DVE stt builds F (2 class chunks, fp32r), two
  accumulated fp32r matmuls per 128-step slab produce mem[t,b] tiles in
  [t-partition, batch-free] layout, evacuated and DMA'd as contiguous
  2KB-per-partition writes.

Engine budget per core (TimelineSim): DMA ~96us (binding: x in 71.4 +
mem out 23.2), PE ~77, DVE/ACT/Pool ~40-55 each.
"""
import sys
if "/opt/trn_rl_repo" not in sys.path:
    sys.path.insert(0, "/opt/trn_rl_repo")

import numpy as np
from contextlib import ExitStack

import concourse.bass as bass
import concourse.bacc as bacc
import concourse.mybir as mybir
import concourse.tile as tile
from concourse.bass_utils import run_bass_kernel_spmd

F32 = mybir.dt.float32
F32R = mybir.dt.float32r
ALU = mybir.AluOpType

N_CORES = 8
B_FULL = 65536
B_CORE = B_FULL // N_CORES          # 8192
D = 784
NUM_STEPS = 255
BETA = 0.95
THRESHOLD = 1.0

GROUP = 512                          # batch per group
NGROUP = B_CORE // GROUP             # 16
BLOCK = 4                            # groups per cur-row block
NBLOCK = NGROUP // BLOCK             # 4
CHUNKS = [(0, 128), (128, 128), (256, 128), (384, 128), (512, 128), (640, 128), (768, 16)]
NCLASS = 256                         # class 0 = ramp; class k = first spike at k
TCHUNKS = [(0, 128), (128, 127)]     # step slabs (255 rows)
DEBUG = False


def _build():
    nc = bacc.Bacc("TRN2", target_bir_lowering=False, debug=False,
                   num_devices=N_CORES)
    x_d = nc.dram_tensor("x", [B_CORE, D], F32, kind="ExternalInput")
    w_d = nc.dram_tensor("w", [128, 7], F32, kind="ExternalInput")
    id_d = nc.dram_tensor("ident", [128, 128], F32, kind="ExternalInput")
    g_d = nc.dram_tensor("gtab", [128, 2 * NUM_STEPS], F32R, kind="ExternalInput")
    thr_d = nc.dram_tensor("thr", [128, 2], F32, kind="ExternalInput")
    ones_d = nc.dram_tensor("ones", [1, 128], F32, kind="ExternalInput")
    mem_d = nc.dram_tensor("mem", [NUM_STEPS, B_CORE], F32, kind="ExternalOutput")
    curscratch_d = nc.dram_tensor("curscratch", [B_CORE], F32)
    if DEBUG:
        dbg_curcols_d = nc.dram_tensor("dbg_curcols", [128, 64], F32, kind="ExternalOutput")
        dbg_currow_d = nc.dram_tensor("dbg_currow", [2048], F32, kind="ExternalOutput")
        dbg_bc_d = nc.dram_tensor("dbg_bc", [128, 512], F32, kind="ExternalOutput")
        dbg_f0_d = nc.dram_tensor("dbg_f0", [128, 512], F32, kind="ExternalOutput")
        dbg_f1_d = nc.dram_tensor("dbg_f1", [128, 512], F32, kind="ExternalOutput")

    with tile.TileContext(nc) as tc, ExitStack() as ctx:
        xpool = ctx.enter_context(tc.tile_pool(name="xpool", bufs=2))
        xtpool = ctx.enter_context(tc.tile_pool(name="xtpool", bufs=4))
        rowpool = ctx.enter_context(tc.tile_pool(name="rowpool", bufs=2))
        fpool = ctx.enter_context(tc.tile_pool(name="fpool", bufs=2))
        opool = ctx.enter_context(tc.tile_pool(name="opool", bufs=3))
        const = ctx.enter_context(tc.tile_pool(name="const", bufs=1))
        psxt = ctx.enter_context(tc.tile_pool(name="psxt", bufs=2, space="PSUM"))
        psacc = ctx.enter_context(tc.tile_pool(name="psacc", bufs=2, space="PSUM"))
        psct = ctx.enter_context(tc.tile_pool(name="psct", bufs=1, space="PSUM"))
        psbc = ctx.enter_context(tc.tile_pool(name="psbc", bufs=1, space="PSUM"))
        psgo = ctx.enter_context(tc.tile_pool(name="psgo", bufs=2, space="PSUM"))

        w_t = const.tile([128, 7], F32)
        id_t = const.tile([128, 128], F32)
        g_t = const.tile([128, 2 * NUM_STEPS], F32R)
        thr_t = const.tile([128, 2], F32)
        ones_t = const.tile([1, 128], F32)
        nc.sync.dma_start(w_t[:], w_d[:])
        nc.sync.dma_start(id_t[:], id_d[:])
        nc.sync.dma_start(g_t[:], g_d[:])
        nc.sync.dma_start(thr_t[:], thr_d[:])
        nc.sync.dma_start(ones_t[:], ones_d[:])

        cur_cols = const.tile([128, NGROUP * 4], F32, name="cur_cols")
        curt_sb = const.tile([BLOCK * 4, 128], F32, name="curt_sb")

        # round-robin PSUM->SBUF copy engines (GPSIMD cannot access PSUM)
        copy_engines = [nc.vector.tensor_copy, nc.scalar.copy]
        copy_idx = [0]

        def copy(out, in_):
            eng = copy_engines[copy_idx[0] % len(copy_engines)]
            copy_idx[0] += 1
            eng(out, in_)

        def group_matvec(g):
            """cur for batches [512g, 512(g+1)) -> cur_cols[:, 4g:4g+4]."""
            xg = []
            for j in range(4):
                xt_ = xpool.tile([128, D], F32, tag=f"xg{j}")
                nc.sync.dma_start(xt_[:], x_d[(4 * g + j) * 128:(4 * g + j + 1) * 128, :])
                xg.append(xt_)
            # A start=True matmul resets its whole PSUM bank, so the four
            # per-column accumulation groups cannot each use start=True:
            # zero the bank once and accumulate with start=False throughout.
            acc = psacc.tile([128, 4], F32, tag="acc")
            nc.vector.memset(acc[:, :], 0.0)
            for ci, (c0, cl) in enumerate(CHUNKS):
                xt_ps = psxt.tile([128, GROUP], F32, tag="xt")
                for j in range(4):
                    nc.tensor.transpose(
                        xt_ps[:cl, j * 128:(j + 1) * 128],
                        xg[j][:, c0:c0 + cl],
                        id_t[:],
                    )
                xt_sb = xtpool.tile([128, GROUP], F32, tag="xtsb")
                copy(xt_sb[:cl, :], xt_ps[:cl, :])
                for j in range(4):
                    nc.tensor.matmul(
                        acc[:, j:j + 1],
                        xt_sb[:cl, j * 128:(j + 1) * 128],
                        w_t[:cl, ci:ci + 1],
                        start=False,
                        stop=(ci == len(CHUNKS) - 1),
                    )
            nc.vector.tensor_copy(cur_cols[:, 4 * g:4 * g + 4], acc[:, :])

        def block_tail(b):
            """Closed-form mem for groups [4b, 4b+4)."""
            ct_ps = psct.tile([BLOCK * 4, 128], F32, tag="ct")
            nc.tensor.transpose(
                ct_ps[:, :], cur_cols[:, 16 * b:16 * (b + 1)], id_t[:])
            nc.vector.tensor_copy(curt_sb[:, :], ct_ps[:, :])
            sl = curscratch_d[2048 * b:2048 * (b + 1)]
            nc.sync.dma_start(sl, curt_sb[:, :])
            cur_row = rowpool.tile([1, BLOCK * GROUP], F32, tag="row")
            nc.sync.dma_start(cur_row[:, :], sl)
            if DEBUG and b == 0:
                nc.sync.dma_start(dbg_currow_d[:], cur_row[0, :])
            for gi in range(BLOCK):
                gg = BLOCK * b + gi
                bc_ps = psbc.tile([128, GROUP], F32, tag="bc")
                nc.tensor.matmul(
                    bc_ps[:, :], ones_t[:, :],
                    cur_row[0:1, gi * GROUP:(gi + 1) * GROUP],
                    start=True, stop=True)
                bc_sb = fpool.tile([128, GROUP], F32, tag="bc_sb")
                nc.scalar.copy(bc_sb[:, :], bc_ps[:, :])
                if DEBUG and gg == 0:
                    nc.sync.dma_start(dbg_bc_d[:], bc_sb[:, :])
                fts = []
                for c in range(2):
                    ft = fpool.tile([128, GROUP], F32R, tag=f"f{c}")
                    nc.vector.scalar_tensor_tensor(
                        ft[:, :], bc_sb[:, :], thr_t[:, c:c + 1], bc_sb[:, :],
                        ALU.is_gt, ALU.mult)
                    fts.append(ft)
                if DEBUG and gg == 0:
                    nc.sync.dma_start(dbg_f0_d[:], fts[0][:, :].bitcast(F32))
                    nc.sync.dma_start(dbg_f1_d[:], fts[1][:, :].bitcast(F32))
                for (t0, tl) in TCHUNKS:
                    go_ps = psgo.tile([128, GROUP], F32, tag="go")
                    for c in range(2):
                        nc.tensor.matmul(
                            go_ps[:tl, :],
                            g_t[:, c * NUM_STEPS + t0:c * NUM_STEPS + t0 + tl],
                            fts[c][:, :],
                            start=(c == 0), stop=(c == 1))
                    o_sb = opool.tile([128, GROUP], F32, tag="osb")
                    copy(o_sb[:tl, :], go_ps[:tl, :])
                    nc.sync.dma_start(
                        mem_d[t0:t0 + tl, gg * GROUP:(gg + 1) * GROUP],
                        o_sb[:tl, :])

        for g in range(NGROUP):
            group_matvec(g)
            if g % BLOCK == BLOCK - 1:
                block_tail(g // BLOCK)
        if DEBUG:
            nc.sync.dma_start(dbg_curcols_d[:], cur_cols[:, :])

    nc.compile()
    return nc


_NC_CACHE = None


def _get_nc():
    global _NC_CACHE
    if _NC_CACHE is None:
        _NC_CACHE = _build()
    return _NC_CACHE


def _round11(a):
    """Round-to-nearest-even at 11 explicit mantissa bits (fp32r grid)."""
    u = np.ascontiguousarray(a, np.float32).view(np.uint32)
    u = (u + 0x800) & 0xFFFFF000
    return u.view(np.float32)


def _host_tables():
    s = np.zeros(NUM_STEPS + 2)
    for k in range(1, NUM_STEPS + 2):
        s[k] = s[k - 1] * BETA + 1.0
    t = np.arange(1, NUM_STEPS + 1)
    R = s[t]

    def pattern(k):
        P = k + 1
        phi = ((t - 1) % P) + 1
        v = s[phi].copy()
        v[phi == P] = 0.0
        return v

    G = np.zeros((NCLASS, NUM_STEPS))
    G[0] = R
    for k in range(1, NCLASS):
        Ak = pattern(k)
        Ak1 = pattern(k + 1) if k + 1 < NCLASS else R
        G[k] = Ak - Ak1
    # gtab layout: [128 partitions, 2 chunks * 255] , class = c*128 + p
    gtab = np.zeros((128, 2 * NUM_STEPS), np.float32)
    for c in range(2):
        gtab[:, c * NUM_STEPS:(c + 1) * NUM_STEPS] = G[c * 128:(c + 1) * 128]
    gtab = _round11(gtab)

    thr = np.zeros((128, 2), np.float32)
    theta = (1.0 / s[1:NCLASS]).astype(np.float32)  # theta_k, k=1..255
    flat = np.concatenate([[np.float32(-3.0e38)], theta])
    thr[:, 0] = flat[0:128]
    thr[:, 1] = flat[128:256]
    return gtab, thr


def _prep_inputs(x, W):
    x = np.ascontiguousarray(np.asarray(x, dtype=np.float32))
    W = np.asarray(W, dtype=np.float32).reshape(-1)
    assert x.shape == (B_FULL, D) and W.shape == (D,)
    wpad = np.zeros(896, np.float32)
    wpad[:D] = W
    wcol = np.ascontiguousarray(wpad.reshape(7, 128).T)
    ident = np.eye(128, dtype=np.float32)
    gtab, thr = _host_tables()
    ones = np.ones((1, 128), np.float32)
    in_maps = [
        {"x": x[d * B_CORE:(d + 1) * B_CORE], "w": wcol, "ident": ident,
         "gtab": gtab, "thr": thr, "ones": ones}
        for d in range(N_CORES)
    ]
    return in_maps


def kernel(x, W, _trace=False, _trace_kwargs=None):
    nc = _get_nc()
    in_maps = _prep_inputs(x, W)
    res = run_bass_kernel_spmd(nc, in_maps, list(range(N_CORES)),
                               trace=_trace, **(_trace_kwargs or {}))
    mem = np.concatenate([res.results[d]["mem"] for d in range(N_CORES)], axis=1)
    mem_rec = mem.reshape(NUM_STEPS, B_FULL, 1)
    spk_rec = (mem_rec > np.float32(THRESHOLD)).astype(np.float32)
    if _trace:
        return (spk_rec, mem_rec), res
    return spk_rec, mem_rec
